# revision 14
# baseline (speedup 1.0000x reference)
"""Trainium2 Bass kernel for BertInfiniSelfAttention.

Math (per batch b):
  q/k/v = hidden @ W{q,k,v} + b       -> split into 12 heads of 64
  kc[h] = mem_keys[h] @ k[:,h]        -> concat over h: [9216, 64]
  vc[h] = mem_values[h] @ v[:,h]      -> concat over h: [9216, 64]
  scores = q_h @ kc.T / 8 + mask      -> softmax over 9216 kv
  ctx_h  = probs @ vc ;  out = sigmoid(gate)_h * ctx

Sharding: each of the 8 cores owns a 1152-row slice of the concatenated
KV axis.  Every core computes (for all 24 (b, head) pairs) the partial
unnormalized numerator  sum_kv exp(s/8)*em_kv*vc  and the partial
denominator sum_kv exp(s/8)*em_kv where em = exp(mask).  The host sums
the partials over cores and divides (flash-attention style combine, no
max subtraction -- scores are O(1) here so exp cannot overflow).

Layouts on device (scores kept transposed [kv, s] throughout):
  qT[b]  [64, 12*768]   head-major, d on partitions
  kv3[b] [128, 6*384]   s-tile major, [k third0..2 | v third0..2] cols
  kcT[b] [64, 1152]     d on partitions, local kv on free
  vca[b] [128, 9*65]    per kv-tile: 64 cols of em-scaled vc + 1 col em
  scoresT psum [128(kv), 768(s)], exp on ACT -> probsT sbuf, ctx via
  lhsT=vca (stationary) x probsT (moving) accumulated over kv tiles.
"""

import numpy as np

B, S, H, NH, D = 2, 768, 768, 12, 64
P = 128
NCORES = 8
KVG = NH * S            # 9216 global kv
KVL = KVG // NCORES     # 1152 local kv per core
NKT = KVL // P          # 9 kv tiles
NT = S // P             # 6 s/H tiles
DP1 = D + 1

_PROGRAM = None
TRACE = False
LAST_RESULTS = None


def _bank_pieces(lo, hi):
    """Split [lo,hi) free-dim range at 512-fp32 PSUM bank boundaries."""
    out = []
    while lo < hi:
        nxt = min(hi, (lo // 512 + 1) * 512)
        out.append((lo, nxt))
        lo = nxt
    return out


def _build_program():
    from contextlib import ExitStack

    import concourse.bacc as bacc
    import concourse.mybir as mybir
    import concourse.tile as tile

    F32 = mybir.dt.float32
    F32R = mybir.dt.float32r
    F16 = mybir.dt.float16
    EXP = mybir.ActivationFunctionType.Exp

    nc = bacc.Bacc("TRN2", target_bir_lowering=False, debug=False,
                   num_devices=NCORES)

    hT = nc.declare_dram_parameter("hT", [B, H, S], F16, isOutput=False)
    wq = nc.declare_dram_parameter("wq", [H, H], F16, isOutput=False)
    bq_d = nc.declare_dram_parameter("bq_d", [P, NH // 2], F32, isOutput=False)
    wkv = nc.declare_dram_parameter("wkv", [H, 6 * D], F16, isOutput=False)
    bkv_d = nc.declare_dram_parameter("bkv_d", [1, 6 * D], F16, isOutput=False)
    mkt = nc.declare_dram_parameter("mkt", [S, KVL], F16, isOutput=False)
    mvt = nc.declare_dram_parameter("mvt", [S, KVL], F16, isOutput=False)
    em_d = nc.declare_dram_parameter("em_d", [B, P, NKT], F32, isOutput=False)
    ones_d = nc.declare_dram_parameter("ones_d", [1, S], F16, isOutput=False)
    out_d = nc.declare_dram_parameter("out_d", [B * NH, DP1, S], F32, isOutput=True)

    with tile.TileContext(nc) as tc, ExitStack() as ctx:
        const = ctx.enter_context(tc.tile_pool(name="const", bufs=1))

        qTa = [const.tile([D, NH * S // 2], F16, name=f"qTa{b}") for b in range(B)]
        qTb = [const.tile([P, NH * S // 2], F16, name=f"qTb{b}") for b in range(B)]
        kv3 = [const.tile([P, NT * 6 * D], F16, name=f"kv3{b}") for b in range(B)]
        kcT = [const.tile([P, KVL], F16, name=f"kcT{b}") for b in range(B)]
        vca = [const.tile([P, NKT * DP1], F16, name=f"vca{b}") for b in range(B)]
        em_s = const.tile([P, B * NKT], F32, name="em_s")
        ones = const.tile([1, S], F16, name="ones")
        bq_s = const.tile([P, NH // 2], F32, name="bq_s")
        bkv_s = const.tile([1, 6 * D], F16, name="bkv_s")

        mkt_s = const.tile([P, NT * KVL], F16, name="mkt_s")
        mvt_s = const.tile([P, NT * KVL], F16, name="mvt_s")
        nc.sync.dma_start(ones[:], ones_d[:])
        for b in range(B):
            nc.sync.dma_start(em_s[:, b * NKT:(b + 1) * NKT], em_d[b])
        nc.sync.dma_start(bq_s[:], bq_d[:])
        nc.sync.dma_start(bkv_s[:], bkv_d[:])

        # ---- Phase A: projections ----
        with tc.tile_pool(name="pa", bufs=1) as pa, \
             tc.tile_pool(name="paps", bufs=2, space="PSUM") as paps:
            wq_s = pa.tile([P, NT * H], F16, name="wq_s")
            wkv_s = pa.tile([P, NT * 6 * D], F16, name="wkv_s")
            nc.sync.dma_start(
                wq_s[:].rearrange("p (kt c) -> p kt c", c=H),
                wq[:].rearrange("(kt p) c -> p kt c", p=P))
            nc.sync.dma_start(
                wkv_s[:].rearrange("p (kt c) -> p kt c", c=6 * D),
                wkv[:].rearrange("(kt p) c -> p kt c", p=P))
            nc.sync.dma_start(
                mkt_s[:].rearrange("p (st c) -> p st c", c=KVL),
                mkt[:].rearrange("(st p) c -> p st c", p=P))
            nc.sync.dma_start(
                mvt_s[:].rearrange("p (st c) -> p st c", c=KVL),
                mvt[:].rearrange("(st p) c -> p st c", p=P))
            for b in range(B):
                hT_s = pa.tile([P, NT * S], F16, name="hT_s", tag="hT_s", bufs=2)
                nc.sync.dma_start(
                    hT_s[:].rearrange("p (kt c) -> p kt c", c=S),
                    hT[b].rearrange("(kt p) c -> p kt c", p=P))
                # qT packed: out [128, 768] = two heads (2t, 2t+1) stacked
                for t in range(NH // 2):
                    q_ps = paps.tile([P, S], F32, name="q_ps", tag="q_ps")
                    for lo, hi in _bank_pieces(0, S):
                        for kt in range(NT):
                            nc.tensor.matmul(
                                q_ps[:, lo:hi],
                                wq_s[:, kt * H + 2 * t * D: kt * H + (2 * t + 2) * D],
                                hT_s[:, kt * S + lo: kt * S + hi],
                                start=(kt == 0), stop=(kt == NT - 1))
                    nc.vector.tensor_scalar_add(
                        qTa[b][:, t * S:(t + 1) * S], q_ps[0:D, :],
                        bq_s[0:D, t:t + 1])
                    nc.vector.tensor_scalar_add(
                        qTb[b][D:P, t * S:(t + 1) * S], q_ps[D:P, :],
                        bq_s[D:P, t:t + 1])
                # k/v thirds: out [128(s), 384] = hT_tile.T @ wkv + ones x bkv
                for st in range(NT):
                    kv_ps = paps.tile([P, 6 * D], F32, name="kv_ps", tag="kv_ps")
                    for kt in range(NT):
                        nc.tensor.matmul(
                            kv_ps[:],
                            hT_s[:, kt * S + st * P: kt * S + (st + 1) * P],
                            wkv_s[:, kt * 6 * D:(kt + 1) * 6 * D],
                            start=(kt == 0), stop=False)
                    nc.tensor.matmul(kv_ps[:], ones[:, 0:P], bkv_s[:],
                                     start=False, stop=True)
                    nc.vector.tensor_copy(kv3[b][:, st * 6 * D:(st + 1) * 6 * D], kv_ps[:])

        # ---- Phase B: memory matmuls ----
        with tc.tile_pool(name="pb", bufs=1) as pb, \
             tc.tile_pool(name="pbps", bufs=2, space="PSUM") as pbps:
            for b in range(B):
                # kcT [128, 1152]: same values in both partition halves so
                # scores lhsT/rhs base partitions can match for odd heads
                kc_ps = pbps.tile([P, KVL], F32, name="kc_ps", tag="kc_ps")
                for j in range(3):
                    for lo, hi in _bank_pieces(384 * j, 384 * (j + 1)):
                        for half, tp in ((0, None), (D, (0, D))):
                            for st in range(NT):
                                nc.tensor.matmul(
                                    kc_ps[half:half + D, lo:hi],
                                    kv3[b][:, st * 6 * D + j * D: st * 6 * D + (j + 1) * D],
                                    mkt_s[:, st * KVL + lo: st * KVL + hi],
                                    start=(st == 0), stop=(st == NT - 1),
                                    tile_position=tp)
                nc.vector.tensor_copy(kcT[b][:], kc_ps[:])
                # vc per kv tile [128, 64], em-scaled into vca
                for t in range(NKT):
                    j = t // 3
                    vc_ps = pbps.tile([P, D], F32, name="vc_ps", tag="vc_ps")
                    for st in range(NT):
                        nc.tensor.matmul(
                            vc_ps[:],
                            mvt_s[:, st * KVL + t * P: st * KVL + (t + 1) * P],
                            kv3[b][:, st * 6 * D + (3 + j) * D: st * 6 * D + (4 + j) * D],
                            start=(st == 0), stop=(st == NT - 1))
                    emc = em_s[:, b * NKT + t: b * NKT + t + 1]
                    nc.vector.tensor_scalar_mul(
                        vca[b][:, t * DP1: t * DP1 + D], vc_ps[:], emc)
                    nc.vector.tensor_copy(
                        vca[b][:, t * DP1 + D: (t + 1) * DP1], emc)

        # ---- Phase C: attention ----
        groups = [(0, 2), (2, 4), (4, 6), (6, 8), (8, 9)]
        with tc.tile_pool(name="pcp", bufs=6) as pcp, \
             tc.tile_pool(name="scps", bufs=2, space="PSUM") as scps, \
             tc.tile_pool(name="ctxps", bufs=1, space="PSUM") as ctxps, \
             tc.tile_pool(name="stg", bufs=2) as stg:
            for b in range(B):
                for qh in range(NH):
                    pair = b * NH + qh
                    ctx_ps = ctxps.tile([DP1, S], F32, name="ctx_ps", tag="ctx")
                    half = D * (qh % 2)
                    qsrc = qTa[b] if qh % 2 == 0 else qTb[b]
                    qcol = (qh // 2) * S
                    # all scores groups + exps first (no PE head-of-line
                    # blocking on exp), then all ctx matmuls
                    prs = []
                    for t0, t1 in groups:
                        w = (t1 - t0) * S
                        sc = scps.tile([P, 2 * S], F32, name="sc", tag="sc")
                        pr = pcp.tile([P, 2 * S], F16, name="pr", tag="pr")
                        for t in range(t0, t1):
                            base = (t - t0) * S
                            for lo, hi in _bank_pieces(base, base + S):
                                nc.tensor.matmul(
                                    sc[:, lo:hi],
                                    kcT[b][half:half + D, t * P:(t + 1) * P],
                                    qsrc[half:half + D, qcol + lo - base: qcol + hi - base],
                                    start=True, stop=True)
                        nc.scalar.activation(pr[:, 0:w], sc[:, 0:w], EXP, scale=0.125)
                        prs.append((t0, t1, pr))
                    for t0, t1, pr in prs:
                        for t in range(t0, t1):
                            base = (t - t0) * S
                            for lo, hi in _bank_pieces(0, S):
                                nc.tensor.matmul(
                                    ctx_ps[:, lo:hi],
                                    vca[b][:, t * DP1:(t + 1) * DP1],
                                    pr[:, base + lo: base + hi],
                                    start=(t == 0), stop=(t == NKT - 1))
                    st_t = stg.tile([DP1, S], F32, name="st_t", tag="st")
                    nc.vector.tensor_copy(st_t[:], ctx_ps[:])
                    nc.sync.dma_start(out_d[pair], st_t[:])

    nc.compile()
    return nc


def _get_program():
    global _PROGRAM
    if _PROGRAM is None:
        _PROGRAM = _build_program()
    return _PROGRAM


def kernel(hidden_states, attention_mask, Wq, bq, Wk, bk, Wv, bv, gate,
           mem_keys, mem_values):
    from concourse.bass_utils import run_bass_kernel_spmd

    global LAST_RESULTS

    f32 = np.float32
    hidden_states = np.asarray(hidden_states, f32)
    attention_mask = np.asarray(attention_mask, f32)
    Wq = np.asarray(Wq, f32)
    bq = np.asarray(bq, f32)
    Wk = np.asarray(Wk, f32)
    bk = np.asarray(bk, f32)
    Wv = np.asarray(Wv, f32)
    bv = np.asarray(bv, f32)
    gate = np.asarray(gate, f32)
    mem_keys = np.asarray(mem_keys, f32)
    mem_values = np.asarray(mem_values, f32)

    hT = np.ascontiguousarray(hidden_states.transpose(0, 2, 1))
    # MKT[s, h*768+kv] = mem_keys[h, kv, s]
    MKT = np.ascontiguousarray(mem_keys.transpose(2, 0, 1).reshape(S, KVG))
    MVT = np.ascontiguousarray(mem_values.transpose(2, 0, 1).reshape(S, KVG))
    mask = attention_mask.reshape(B, KVG)
    em_full = np.exp(mask).astype(f32)
    hT16 = hT.astype(np.float16)
    Wq16 = Wq.astype(np.float16)

    in_maps = []
    for c in range(NCORES):
        kvb = c * KVL
        heads = [(kvb + 384 * j) // S for j in range(3)]
        wkv_c = np.concatenate(
            [Wk[:, h * D:(h + 1) * D] for h in heads]
            + [Wv[:, h * D:(h + 1) * D] for h in heads], axis=1)
        bkv_c = np.concatenate(
            [bk[h * D:(h + 1) * D] for h in heads]
            + [bv[h * D:(h + 1) * D] for h in heads])[None, :]
        em_c = em_full[:, kvb:kvb + KVL].reshape(B, NKT, P).transpose(0, 2, 1)
        f16 = np.float16
        in_maps.append({
            "hT": hT16,
            "wq": Wq16,
            "bq_d": np.ascontiguousarray(
                bq.reshape(NH // 2, 2, D).transpose(1, 2, 0).reshape(P, NH // 2)),
            "wkv": np.ascontiguousarray(wkv_c).astype(f16),
            "bkv_d": np.ascontiguousarray(bkv_c).astype(f16),
            "mkt": np.ascontiguousarray(MKT[:, kvb:kvb + KVL]).astype(f16),
            "mvt": np.ascontiguousarray(MVT[:, kvb:kvb + KVL]).astype(f16),
            "em_d": np.ascontiguousarray(em_c),
            "ones_d": np.ones((1, S), f16),
        })

    nc = _get_program()
    res = run_bass_kernel_spmd(nc, in_maps, core_ids=list(range(NCORES)),
                               trace=TRACE)
    LAST_RESULTS = res

    parts = res.results[0]["out_d"].astype(f32).copy()
    for rr in res.results[1:]:
        parts += rr["out_d"]
    num = parts[:, :D, :]                      # [24, 64, 768]
    den = parts[:, D, :]                       # [24, 768]
    ctxT = num / den[:, None, :]
    ctx = ctxT.reshape(B, NH, D, S).transpose(0, 3, 1, 2)   # [B, S, NH, D]
    g = (1.0 / (1.0 + np.exp(-gate))).reshape(1, 1, NH, 1)
    return (g * ctx).astype(f32)


# revision 15
# speedup vs baseline: 1.0388x; 1.0388x over previous
"""Trainium2 Bass kernel for BertInfiniSelfAttention.

Math (per batch b):
  q/k/v = hidden @ W{q,k,v} + b       -> split into 12 heads of 64
  kc[h] = mem_keys[h] @ k[:,h]        -> concat over h: [9216, 64]
  vc[h] = mem_values[h] @ v[:,h]      -> concat over h: [9216, 64]
  scores = q_h @ kc.T / 8 + mask      -> softmax over 9216 kv
  ctx_h  = probs @ vc ;  out = sigmoid(gate)_h * ctx

Sharding: each of the 8 cores owns a 1152-row slice of the concatenated
KV axis.  Every core computes (for all 24 (b, head) pairs) the partial
unnormalized numerator  sum_kv exp(s/8)*em_kv*vc  and the partial
denominator sum_kv exp(s/8)*em_kv where em = exp(mask).  The host sums
the partials over cores and divides (flash-attention style combine, no
max subtraction -- scores are O(1) here so exp cannot overflow).

Layouts on device (scores kept transposed [kv, s] throughout):
  qT[b]  [64, 12*768]   head-major, d on partitions
  kv3[b] [128, 6*384]   s-tile major, [k third0..2 | v third0..2] cols
  kcT[b] [64, 1152]     d on partitions, local kv on free
  vca[b] [128, 9*65]    per kv-tile: 64 cols of em-scaled vc + 1 col em
  scoresT psum [128(kv), 768(s)], exp on ACT -> probsT sbuf, ctx via
  lhsT=vca (stationary) x probsT (moving) accumulated over kv tiles.
"""

import numpy as np

B, S, H, NH, D = 2, 768, 768, 12, 64
P = 128
NCORES = 8
KVG = NH * S            # 9216 global kv
KVL = KVG // NCORES     # 1152 local kv per core
NKT = KVL // P          # 9 kv tiles
NT = S // P             # 6 s/H tiles
DP1 = D + 1

_PROGRAM = None
TRACE = False
LAST_RESULTS = None


def _bank_pieces(lo, hi):
    """Split [lo,hi) free-dim range at 512-fp32 PSUM bank boundaries."""
    out = []
    while lo < hi:
        nxt = min(hi, (lo // 512 + 1) * 512)
        out.append((lo, nxt))
        lo = nxt
    return out


def _build_program():
    from contextlib import ExitStack

    import concourse.bacc as bacc
    import concourse.mybir as mybir
    import concourse.tile as tile

    F32 = mybir.dt.float32
    F32R = mybir.dt.float32r
    F16 = mybir.dt.float16
    EXP = mybir.ActivationFunctionType.Exp

    nc = bacc.Bacc("TRN2", target_bir_lowering=False, debug=False,
                   num_devices=NCORES)

    hT = nc.declare_dram_parameter("hT", [B, H, S], F16, isOutput=False)
    wq = nc.declare_dram_parameter("wq", [H, H], F16, isOutput=False)
    bq_d = nc.declare_dram_parameter("bq_d", [P, NH // 2], F32, isOutput=False)
    wkv = nc.declare_dram_parameter("wkv", [H, 6 * D], F16, isOutput=False)
    bkv_d = nc.declare_dram_parameter("bkv_d", [1, 6 * D], F16, isOutput=False)
    mkt = nc.declare_dram_parameter("mkt", [S, KVL], F16, isOutput=False)
    mvt = nc.declare_dram_parameter("mvt", [S, KVL], F16, isOutput=False)
    em_d = nc.declare_dram_parameter("em_d", [B, P, NKT], F32, isOutput=False)
    ones_d = nc.declare_dram_parameter("ones_d", [1, S], F16, isOutput=False)
    out_d = nc.declare_dram_parameter("out_d", [B * NH, DP1, S], F32, isOutput=True)

    with tile.TileContext(nc) as tc, ExitStack() as ctx:
        const = ctx.enter_context(tc.tile_pool(name="const", bufs=1))

        qTa = [const.tile([D, NH * S // 2], F16, name=f"qTa{b}") for b in range(B)]
        qTb = [const.tile([P, NH * S // 2], F16, name=f"qTb{b}") for b in range(B)]
        kv3 = [const.tile([P, NT * 6 * D], F16, name=f"kv3{b}") for b in range(B)]
        kcT = [const.tile([P, KVL], F16, name=f"kcT{b}") for b in range(B)]
        vca = [const.tile([P, NKT * DP1], F16, name=f"vca{b}") for b in range(B)]
        em_s = const.tile([P, B * NKT], F32, name="em_s")
        ones = const.tile([1, S], F16, name="ones")
        bq_s = const.tile([P, NH // 2], F32, name="bq_s")
        bkv_s = const.tile([1, 6 * D], F16, name="bkv_s")

        mkt_s = const.tile([P, NT * KVL], F16, name="mkt_s")
        mvt_s = const.tile([P, NT * KVL], F16, name="mvt_s")
        nc.gpsimd.dma_start(ones[:], ones_d[:])
        for b in range(B):
            nc.gpsimd.dma_start(em_s[:, b * NKT:(b + 1) * NKT], em_d[b])
        nc.gpsimd.dma_start(bq_s[:], bq_d[:])
        nc.gpsimd.dma_start(bkv_s[:], bkv_d[:])

        # ---- Phase A: projections ----
        with tc.tile_pool(name="pa", bufs=1) as pa, \
             tc.tile_pool(name="paps", bufs=2, space="PSUM") as paps:
            wq_s = pa.tile([P, NT * H], F16, name="wq_s")
            wkv_s = pa.tile([P, NT * 6 * D], F16, name="wkv_s")
            hT_tiles = [pa.tile([P, NT * S], F16, name=f"hT_s{b}") for b in range(B)]
            nc.sync.dma_start(
                wq_s[:].rearrange("p (kt c) -> p kt c", c=H),
                wq[:].rearrange("(kt p) c -> p kt c", p=P))
            nc.sync.dma_start(
                hT_tiles[0][:].rearrange("p (kt c) -> p kt c", c=S),
                hT[0].rearrange("(kt p) c -> p kt c", p=P))
            nc.sync.dma_start(
                wkv_s[:].rearrange("p (kt c) -> p kt c", c=6 * D),
                wkv[:].rearrange("(kt p) c -> p kt c", p=P))
            nc.sync.dma_start(
                hT_tiles[1][:].rearrange("p (kt c) -> p kt c", c=S),
                hT[1].rearrange("(kt p) c -> p kt c", p=P))
            for st in range(NT):
                nc.sync.dma_start(mkt_s[:, st * KVL:(st + 1) * KVL],
                                  mkt[st * P:(st + 1) * P, :])
                nc.sync.dma_start(mvt_s[:, st * KVL:(st + 1) * KVL],
                                  mvt[st * P:(st + 1) * P, :])
            for b in range(B):
                hT_s = hT_tiles[b]
                # qT packed: out [128, 768] = two heads (2t, 2t+1) stacked
                for t in range(NH // 2):
                    q_ps = paps.tile([P, S], F32, name="q_ps", tag="q_ps")
                    for lo, hi in _bank_pieces(0, S):
                        for kt in range(NT):
                            nc.tensor.matmul(
                                q_ps[:, lo:hi],
                                wq_s[:, kt * H + 2 * t * D: kt * H + (2 * t + 2) * D],
                                hT_s[:, kt * S + lo: kt * S + hi],
                                start=(kt == 0), stop=(kt == NT - 1))
                    nc.vector.tensor_scalar_add(
                        qTa[b][:, t * S:(t + 1) * S], q_ps[0:D, :],
                        bq_s[0:D, t:t + 1])
                    nc.vector.tensor_scalar_add(
                        qTb[b][D:P, t * S:(t + 1) * S], q_ps[D:P, :],
                        bq_s[D:P, t:t + 1])
                # k/v thirds: out [128(s), 384] = hT_tile.T @ wkv + ones x bkv
                for st in range(NT):
                    kv_ps = paps.tile([P, 6 * D], F32, name="kv_ps", tag="kv_ps")
                    for kt in range(NT):
                        nc.tensor.matmul(
                            kv_ps[:],
                            hT_s[:, kt * S + st * P: kt * S + (st + 1) * P],
                            wkv_s[:, kt * 6 * D:(kt + 1) * 6 * D],
                            start=(kt == 0), stop=False)
                    nc.tensor.matmul(kv_ps[:], ones[:, 0:P], bkv_s[:],
                                     start=False, stop=True)
                    nc.vector.tensor_copy(kv3[b][:, st * 6 * D:(st + 1) * 6 * D], kv_ps[:])

        # ---- Phase B: memory matmuls ----
        with tc.tile_pool(name="pb", bufs=1) as pb, \
             tc.tile_pool(name="pbps", bufs=2, space="PSUM") as pbps:
            for b in range(B):
                # kcT [128, 1152]: same values in both partition halves so
                # scores lhsT/rhs base partitions can match for odd heads
                kc_ps = pbps.tile([P, KVL], F32, name="kc_ps", tag="kc_ps")
                for j in range(3):
                    for lo, hi in _bank_pieces(384 * j, 384 * (j + 1)):
                        for half, tp in ((0, None), (D, (0, D))):
                            for st in range(NT):
                                nc.tensor.matmul(
                                    kc_ps[half:half + D, lo:hi],
                                    kv3[b][:, st * 6 * D + j * D: st * 6 * D + (j + 1) * D],
                                    mkt_s[:, st * KVL + lo: st * KVL + hi],
                                    start=(st == 0), stop=(st == NT - 1),
                                    tile_position=tp)
                nc.vector.tensor_copy(kcT[b][:], kc_ps[:])
                # vc per kv tile [128, 64], em-scaled into vca
                for t in range(NKT):
                    j = t // 3
                    vc_ps = pbps.tile([P, D], F32, name="vc_ps", tag="vc_ps")
                    for st in range(NT):
                        nc.tensor.matmul(
                            vc_ps[:],
                            mvt_s[:, st * KVL + t * P: st * KVL + (t + 1) * P],
                            kv3[b][:, st * 6 * D + (3 + j) * D: st * 6 * D + (4 + j) * D],
                            start=(st == 0), stop=(st == NT - 1))
                    emc = em_s[:, b * NKT + t: b * NKT + t + 1]
                    nc.vector.tensor_scalar_mul(
                        vca[b][:, t * DP1: t * DP1 + D], vc_ps[:], emc)
                    nc.vector.tensor_copy(
                        vca[b][:, t * DP1 + D: (t + 1) * DP1], emc)

        # ---- Phase C: attention ----
        groups = [(0, 2), (2, 4), (4, 6), (6, 8), (8, 9)]
        with tc.tile_pool(name="pcp", bufs=6) as pcp, \
             tc.tile_pool(name="scps", bufs=2, space="PSUM") as scps, \
             tc.tile_pool(name="ctxps", bufs=1, space="PSUM") as ctxps, \
             tc.tile_pool(name="stg", bufs=2) as stg:
            for b in range(B):
                for qh in range(NH):
                    pair = b * NH + qh
                    ctx_ps = ctxps.tile([DP1, S], F32, name="ctx_ps", tag="ctx")
                    half = D * (qh % 2)
                    qsrc = qTa[b] if qh % 2 == 0 else qTb[b]
                    qcol = (qh // 2) * S
                    # all scores groups + exps first (no PE head-of-line
                    # blocking on exp), then all ctx matmuls
                    prs = []
                    for t0, t1 in groups:
                        w = (t1 - t0) * S
                        sc = scps.tile([P, 2 * S], F32, name="sc", tag="sc")
                        pr = pcp.tile([P, 2 * S], F16, name="pr", tag="pr")
                        for t in range(t0, t1):
                            base = (t - t0) * S
                            for lo, hi in _bank_pieces(base, base + S):
                                nc.tensor.matmul(
                                    sc[:, lo:hi],
                                    kcT[b][half:half + D, t * P:(t + 1) * P],
                                    qsrc[half:half + D, qcol + lo - base: qcol + hi - base],
                                    start=True, stop=True)
                        nc.scalar.activation(pr[:, 0:w], sc[:, 0:w], EXP, scale=0.125)
                        prs.append((t0, t1, pr))
                    for t0, t1, pr in prs:
                        for t in range(t0, t1):
                            base = (t - t0) * S
                            for lo, hi in _bank_pieces(0, S):
                                nc.tensor.matmul(
                                    ctx_ps[:, lo:hi],
                                    vca[b][:, t * DP1:(t + 1) * DP1],
                                    pr[:, base + lo: base + hi],
                                    start=(t == 0), stop=(t == NKT - 1))
                    st_t = stg.tile([DP1, S], F32, name="st_t", tag="st")
                    nc.vector.tensor_copy(st_t[:], ctx_ps[:])
                    nc.sync.dma_start(out_d[pair], st_t[:])

    nc.compile()
    return nc


def _get_program():
    global _PROGRAM
    if _PROGRAM is None:
        _PROGRAM = _build_program()
    return _PROGRAM


def kernel(hidden_states, attention_mask, Wq, bq, Wk, bk, Wv, bv, gate,
           mem_keys, mem_values):
    from concourse.bass_utils import run_bass_kernel_spmd

    global LAST_RESULTS

    f32 = np.float32
    hidden_states = np.asarray(hidden_states, f32)
    attention_mask = np.asarray(attention_mask, f32)
    Wq = np.asarray(Wq, f32)
    bq = np.asarray(bq, f32)
    Wk = np.asarray(Wk, f32)
    bk = np.asarray(bk, f32)
    Wv = np.asarray(Wv, f32)
    bv = np.asarray(bv, f32)
    gate = np.asarray(gate, f32)
    mem_keys = np.asarray(mem_keys, f32)
    mem_values = np.asarray(mem_values, f32)

    hT = np.ascontiguousarray(hidden_states.transpose(0, 2, 1))
    # MKT[s, h*768+kv] = mem_keys[h, kv, s]
    MKT = np.ascontiguousarray(mem_keys.transpose(2, 0, 1).reshape(S, KVG))
    MVT = np.ascontiguousarray(mem_values.transpose(2, 0, 1).reshape(S, KVG))
    mask = attention_mask.reshape(B, KVG)
    em_full = np.exp(mask).astype(f32)
    hT16 = hT.astype(np.float16)
    Wq16 = Wq.astype(np.float16)

    in_maps = []
    for c in range(NCORES):
        kvb = c * KVL
        heads = [(kvb + 384 * j) // S for j in range(3)]
        wkv_c = np.concatenate(
            [Wk[:, h * D:(h + 1) * D] for h in heads]
            + [Wv[:, h * D:(h + 1) * D] for h in heads], axis=1)
        bkv_c = np.concatenate(
            [bk[h * D:(h + 1) * D] for h in heads]
            + [bv[h * D:(h + 1) * D] for h in heads])[None, :]
        em_c = em_full[:, kvb:kvb + KVL].reshape(B, NKT, P).transpose(0, 2, 1)
        f16 = np.float16
        in_maps.append({
            "hT": hT16,
            "wq": Wq16,
            "bq_d": np.ascontiguousarray(
                bq.reshape(NH // 2, 2, D).transpose(1, 2, 0).reshape(P, NH // 2)),
            "wkv": np.ascontiguousarray(wkv_c).astype(f16),
            "bkv_d": np.ascontiguousarray(bkv_c).astype(f16),
            "mkt": np.ascontiguousarray(MKT[:, kvb:kvb + KVL]).astype(f16),
            "mvt": np.ascontiguousarray(MVT[:, kvb:kvb + KVL]).astype(f16),
            "em_d": np.ascontiguousarray(em_c),
            "ones_d": np.ones((1, S), f16),
        })

    nc = _get_program()
    res = run_bass_kernel_spmd(nc, in_maps, core_ids=list(range(NCORES)),
                               trace=TRACE)
    LAST_RESULTS = res

    parts = res.results[0]["out_d"].astype(f32).copy()
    for rr in res.results[1:]:
        parts += rr["out_d"]
    num = parts[:, :D, :]                      # [24, 64, 768]
    den = parts[:, D, :]                       # [24, 768]
    ctxT = num / den[:, None, :]
    ctx = ctxT.reshape(B, NH, D, S).transpose(0, 3, 1, 2)   # [B, S, NH, D]
    g = (1.0 / (1.0 + np.exp(-gate))).reshape(1, 1, NH, 1)
    return (g * ctx).astype(f32)


# revision 16
# speedup vs baseline: 1.2327x; 1.1867x over previous
"""Trainium2 Bass kernel for BertInfiniSelfAttention.

Math (per batch b):
  q/k/v = hidden @ W{q,k,v} + b       -> split into 12 heads of 64
  kc[h] = mem_keys[h] @ k[:,h]        -> concat over h: [9216, 64]
  vc[h] = mem_values[h] @ v[:,h]      -> concat over h: [9216, 64]
  scores = q_h @ kc.T / 8 + mask      -> softmax over 9216 kv
  ctx_h  = probs @ vc ;  out = sigmoid(gate)_h * ctx

Sharding: each of the 8 cores owns a 1152-row slice of the concatenated
KV axis.  Every core computes (for all 24 (b, head) pairs) the partial
unnormalized numerator  sum_kv exp(s/8)*em_kv*vc  and the partial
denominator sum_kv exp(s/8)*em_kv where em = exp(mask).  The host sums
the partials over cores and divides (flash-attention style combine, no
max subtraction -- scores are O(1) here so exp cannot overflow).

Layouts on device (scores kept transposed [kv, s] throughout):
  qT[b]  [64, 12*768]   head-major, d on partitions
  kv3[b] [128, 6*384]   s-tile major, [k third0..2 | v third0..2] cols
  kcT[b] [64, 1152]     d on partitions, local kv on free
  vca[b] [128, 9*65]    per kv-tile: 64 cols of em-scaled vc + 1 col em
  scoresT psum [128(kv), 768(s)], exp on ACT -> probsT sbuf, ctx via
  lhsT=vca (stationary) x probsT (moving) accumulated over kv tiles.
"""

import numpy as np

B, S, H, NH, D = 2, 768, 768, 12, 64
P = 128
NCORES = 8
KVG = NH * S            # 9216 global kv
KVL = KVG // NCORES     # 1152 local kv per core
NKT = KVL // P          # 9 kv tiles
NT = S // P             # 6 s/H tiles
DP1 = D + 1

_PROGRAM = None
TRACE = False
LAST_RESULTS = None


def _bank_pieces(lo, hi):
    """Split [lo,hi) free-dim range at 512-fp32 PSUM bank boundaries."""
    out = []
    while lo < hi:
        nxt = min(hi, (lo // 512 + 1) * 512)
        out.append((lo, nxt))
        lo = nxt
    return out


def _build_program():
    from contextlib import ExitStack

    import concourse.bacc as bacc
    import concourse.mybir as mybir
    import concourse.tile as tile

    F32 = mybir.dt.float32
    F32R = mybir.dt.float32r
    F16 = mybir.dt.float16
    EXP = mybir.ActivationFunctionType.Exp

    nc = bacc.Bacc("TRN2", target_bir_lowering=False, debug=False,
                   num_devices=NCORES)

    hT = nc.declare_dram_parameter("hT", [B, H, S], F16, isOutput=False)
    wq = nc.declare_dram_parameter("wq", [H, H], F16, isOutput=False)
    bq_d = nc.declare_dram_parameter("bq_d", [P, NH // 2], F32, isOutput=False)
    wkv = nc.declare_dram_parameter("wkv", [H, 6 * D], F16, isOutput=False)
    bkv_d = nc.declare_dram_parameter("bkv_d", [1, 6 * D], F16, isOutput=False)
    mkt = nc.declare_dram_parameter("mkt", [S, KVL], F16, isOutput=False)
    mvt = nc.declare_dram_parameter("mvt", [S, KVL], F16, isOutput=False)
    em_d = nc.declare_dram_parameter("em_d", [B, P, NKT], F32, isOutput=False)
    ones_d = nc.declare_dram_parameter("ones_d", [1, S], F16, isOutput=False)
    out_d = nc.declare_dram_parameter("out_d", [B * NH, DP1, S], F32, isOutput=True)

    with tile.TileContext(nc) as tc, ExitStack() as ctx:
        const = ctx.enter_context(tc.tile_pool(name="const", bufs=1))

        qTa = [const.tile([D, NH * S // 2], F16, name=f"qTa{b}") for b in range(B)]
        qTb = [const.tile([P, NH * S // 2], F16, name=f"qTb{b}") for b in range(B)]
        kv3 = [const.tile([P, NT * 6 * D], F16, name=f"kv3{b}") for b in range(B)]
        kcT = [const.tile([P, KVL], F16, name=f"kcT{b}") for b in range(B)]
        vca = [const.tile([P, NKT * DP1], F16, name=f"vca{b}") for b in range(B)]
        em_s = const.tile([P, B * NKT], F32, name="em_s")
        ones = const.tile([1, S], F16, name="ones")
        bq_s = const.tile([P, NH // 2], F32, name="bq_s")
        bkv_s = const.tile([1, 6 * D], F16, name="bkv_s")

        mkt_s = const.tile([P, NT * KVL], F16, name="mkt_s")
        mvt_s = const.tile([P, NT * KVL], F16, name="mvt_s")
        nc.gpsimd.dma_start(ones[:], ones_d[:])
        for b in range(B):
            nc.gpsimd.dma_start(em_s[:, b * NKT:(b + 1) * NKT], em_d[b])
        nc.gpsimd.dma_start(bq_s[:], bq_d[:])
        nc.gpsimd.dma_start(bkv_s[:], bkv_d[:])

        # ---- Phase A: projections ----
        with tc.tile_pool(name="pa", bufs=1) as pa, \
             tc.tile_pool(name="paps", bufs=2, space="PSUM") as paps:
            wq_s = pa.tile([P, NT * H], F16, name="wq_s")
            wkv_s = pa.tile([P, NT * 6 * D], F16, name="wkv_s")
            hT_tiles = [pa.tile([P, NT * S], F16, name=f"hT_s{b}") for b in range(B)]
            nc.sync.dma_start(
                wq_s[:].rearrange("p (kt c) -> p kt c", c=H),
                wq[:].rearrange("(kt p) c -> p kt c", p=P))
            nc.sync.dma_start(
                hT_tiles[0][:].rearrange("p (kt c) -> p kt c", c=S),
                hT[0].rearrange("(kt p) c -> p kt c", p=P))
            nc.sync.dma_start(
                wkv_s[:].rearrange("p (kt c) -> p kt c", c=6 * D),
                wkv[:].rearrange("(kt p) c -> p kt c", p=P))
            nc.sync.dma_start(
                hT_tiles[1][:].rearrange("p (kt c) -> p kt c", c=S),
                hT[1].rearrange("(kt p) c -> p kt c", p=P))
            for st in range(NT):
                nc.sync.dma_start(mkt_s[:, st * KVL:(st + 1) * KVL],
                                  mkt[st * P:(st + 1) * P, :])
                nc.sync.dma_start(mvt_s[:, st * KVL:(st + 1) * KVL],
                                  mvt[st * P:(st + 1) * P, :])
            for b in range(B):
                hT_s = hT_tiles[b]
                # qT packed: out [128, 768] = two heads (2t, 2t+1) stacked
                for t in range(NH // 2):
                    q_ps = paps.tile([P, S], F32, name="q_ps", tag="q_ps")
                    for lo, hi in _bank_pieces(0, S):
                        for kt in range(NT):
                            nc.tensor.matmul(
                                q_ps[:, lo:hi],
                                wq_s[:, kt * H + 2 * t * D: kt * H + (2 * t + 2) * D],
                                hT_s[:, kt * S + lo: kt * S + hi],
                                start=(kt == 0), stop=(kt == NT - 1))
                    nc.vector.tensor_scalar_add(
                        qTa[b][:, t * S:(t + 1) * S], q_ps[0:D, :],
                        bq_s[0:D, t:t + 1])
                    nc.vector.tensor_scalar_add(
                        qTb[b][D:P, t * S:(t + 1) * S], q_ps[D:P, :],
                        bq_s[D:P, t:t + 1])
                # k/v thirds: out [128(s), 384] = hT_tile.T @ wkv + ones x bkv
                for st in range(NT):
                    kv_ps = paps.tile([P, 6 * D], F32, name="kv_ps", tag="kv_ps")
                    for kt in range(NT):
                        nc.tensor.matmul(
                            kv_ps[:],
                            hT_s[:, kt * S + st * P: kt * S + (st + 1) * P],
                            wkv_s[:, kt * 6 * D:(kt + 1) * 6 * D],
                            start=(kt == 0), stop=False)
                    nc.tensor.matmul(kv_ps[:], ones[:, 0:P], bkv_s[:],
                                     start=False, stop=True)
                    nc.vector.tensor_copy(kv3[b][:, st * 6 * D:(st + 1) * 6 * D], kv_ps[:])

        # ---- Phase B: memory matmuls ----
        with tc.tile_pool(name="pb", bufs=1) as pb, \
             tc.tile_pool(name="pbps", bufs=2, space="PSUM") as pbps:
            for b in range(B):
                # kcT [128, 1152]: same values in both partition halves so
                # scores lhsT/rhs base partitions can match for odd heads
                kc_ps = pbps.tile([P, KVL], F32, name="kc_ps", tag="kc_ps")
                for j in range(3):
                    for lo, hi in _bank_pieces(384 * j, 384 * (j + 1)):
                        for half, tp in ((0, None), (D, (0, D))):
                            for st in range(NT):
                                nc.tensor.matmul(
                                    kc_ps[half:half + D, lo:hi],
                                    kv3[b][:, st * 6 * D + j * D: st * 6 * D + (j + 1) * D],
                                    mkt_s[:, st * KVL + lo: st * KVL + hi],
                                    start=(st == 0), stop=(st == NT - 1),
                                    tile_position=tp)
                nc.vector.tensor_copy(kcT[b][:], kc_ps[:])
                # vc per kv tile [128, 64], em-scaled into vca
                for t in range(NKT):
                    j = t // 3
                    vc_ps = pbps.tile([P, D], F32, name="vc_ps", tag="vc_ps")
                    for st in range(NT):
                        nc.tensor.matmul(
                            vc_ps[:],
                            mvt_s[:, st * KVL + t * P: st * KVL + (t + 1) * P],
                            kv3[b][:, st * 6 * D + (3 + j) * D: st * 6 * D + (4 + j) * D],
                            start=(st == 0), stop=(st == NT - 1))
                    emc = em_s[:, b * NKT + t: b * NKT + t + 1]
                    nc.vector.tensor_scalar_mul(
                        vca[b][:, t * DP1: t * DP1 + D], vc_ps[:], emc)
                    nc.vector.tensor_copy(
                        vca[b][:, t * DP1 + D: (t + 1) * DP1], emc)

        # ---- Phase C: attention ----
        groups = [(0, 1), (1, 3), (3, 5), (5, 7), (7, 9)]
        with tc.tile_pool(name="pcp", bufs=6) as pcp, \
             tc.tile_pool(name="scps", bufs=2, space="PSUM") as scps, \
             tc.tile_pool(name="ctxps", bufs=1, space="PSUM") as ctxps, \
             tc.tile_pool(name="stg", bufs=2) as stg:
            for b in range(B):
                for qh in range(NH):
                    pair = b * NH + qh
                    ctx_ps = ctxps.tile([DP1, S], F32, name="ctx_ps", tag="ctx")
                    half = D * (qh % 2)
                    qsrc = qTa[b] if qh % 2 == 0 else qTb[b]
                    qcol = (qh // 2) * S
                    # all scores groups + exps first (no PE head-of-line
                    # blocking on exp), then all ctx matmuls
                    prs = []
                    for t0, t1 in groups:
                        w = (t1 - t0) * S
                        sc = scps.tile([P, 2 * S], F32, name="sc", tag="sc")
                        pr = pcp.tile([P, 2 * S], F16, name="pr", tag="pr")
                        for t in range(t0, t1):
                            base = (t - t0) * S
                            for lo, hi in _bank_pieces(base, base + S):
                                nc.tensor.matmul(
                                    sc[:, lo:hi],
                                    kcT[b][half:half + D, t * P:(t + 1) * P],
                                    qsrc[half:half + D, qcol + lo - base: qcol + hi - base],
                                    start=True, stop=True)
                        nc.scalar.activation(pr[:, 0:w], sc[:, 0:w], EXP, scale=0.125)
                        prs.append((t0, t1, pr))
                    for t0, t1, pr in prs:
                        for t in range(t0, t1):
                            base = (t - t0) * S
                            for lo, hi in _bank_pieces(0, S):
                                nc.tensor.matmul(
                                    ctx_ps[:, lo:hi],
                                    vca[b][:, t * DP1:(t + 1) * DP1],
                                    pr[:, base + lo: base + hi],
                                    start=(t == 0), stop=(t == NKT - 1))
                    st_t = stg.tile([DP1, S], F32, name="st_t", tag="st")
                    nc.vector.tensor_copy(st_t[:], ctx_ps[:])
                    nc.sync.dma_start(out_d[pair], st_t[:])

    nc.compile()
    return nc


def _get_program():
    global _PROGRAM
    if _PROGRAM is None:
        _PROGRAM = _build_program()
    return _PROGRAM


def kernel(hidden_states, attention_mask, Wq, bq, Wk, bk, Wv, bv, gate,
           mem_keys, mem_values):
    from concourse.bass_utils import run_bass_kernel_spmd

    global LAST_RESULTS

    f32 = np.float32
    hidden_states = np.asarray(hidden_states, f32)
    attention_mask = np.asarray(attention_mask, f32)
    Wq = np.asarray(Wq, f32)
    bq = np.asarray(bq, f32)
    Wk = np.asarray(Wk, f32)
    bk = np.asarray(bk, f32)
    Wv = np.asarray(Wv, f32)
    bv = np.asarray(bv, f32)
    gate = np.asarray(gate, f32)
    mem_keys = np.asarray(mem_keys, f32)
    mem_values = np.asarray(mem_values, f32)

    hT = np.ascontiguousarray(hidden_states.transpose(0, 2, 1))
    # MKT[s, h*768+kv] = mem_keys[h, kv, s]
    MKT = np.ascontiguousarray(mem_keys.transpose(2, 0, 1).reshape(S, KVG))
    MVT = np.ascontiguousarray(mem_values.transpose(2, 0, 1).reshape(S, KVG))
    mask = attention_mask.reshape(B, KVG)
    em_full = np.exp(mask).astype(f32)
    hT16 = hT.astype(np.float16)
    Wq16 = Wq.astype(np.float16)

    in_maps = []
    for c in range(NCORES):
        kvb = c * KVL
        heads = [(kvb + 384 * j) // S for j in range(3)]
        wkv_c = np.concatenate(
            [Wk[:, h * D:(h + 1) * D] for h in heads]
            + [Wv[:, h * D:(h + 1) * D] for h in heads], axis=1)
        bkv_c = np.concatenate(
            [bk[h * D:(h + 1) * D] for h in heads]
            + [bv[h * D:(h + 1) * D] for h in heads])[None, :]
        em_c = em_full[:, kvb:kvb + KVL].reshape(B, NKT, P).transpose(0, 2, 1)
        f16 = np.float16
        in_maps.append({
            "hT": hT16,
            "wq": Wq16,
            "bq_d": np.ascontiguousarray(
                bq.reshape(NH // 2, 2, D).transpose(1, 2, 0).reshape(P, NH // 2)),
            "wkv": np.ascontiguousarray(wkv_c).astype(f16),
            "bkv_d": np.ascontiguousarray(bkv_c).astype(f16),
            "mkt": np.ascontiguousarray(MKT[:, kvb:kvb + KVL]).astype(f16),
            "mvt": np.ascontiguousarray(MVT[:, kvb:kvb + KVL]).astype(f16),
            "em_d": np.ascontiguousarray(em_c),
            "ones_d": np.ones((1, S), f16),
        })

    nc = _get_program()
    res = run_bass_kernel_spmd(nc, in_maps, core_ids=list(range(NCORES)),
                               trace=TRACE)
    LAST_RESULTS = res

    parts = res.results[0]["out_d"].astype(f32).copy()
    for rr in res.results[1:]:
        parts += rr["out_d"]
    num = parts[:, :D, :]                      # [24, 64, 768]
    den = parts[:, D, :]                       # [24, 768]
    ctxT = num / den[:, None, :]
    ctx = ctxT.reshape(B, NH, D, S).transpose(0, 3, 1, 2)   # [B, S, NH, D]
    g = (1.0 / (1.0 + np.exp(-gate))).reshape(1, 1, NH, 1)
    return (g * ctx).astype(f32)


# revision 17
# speedup vs baseline: 1.2443x; 1.0094x over previous
"""Trainium2 Bass kernel for BertInfiniSelfAttention.

Math (per batch b):
  q/k/v = hidden @ W{q,k,v} + b       -> split into 12 heads of 64
  kc[h] = mem_keys[h] @ k[:,h]        -> concat over h: [9216, 64]
  vc[h] = mem_values[h] @ v[:,h]      -> concat over h: [9216, 64]
  scores = q_h @ kc.T / 8 + mask      -> softmax over 9216 kv
  ctx_h  = probs @ vc ;  out = sigmoid(gate)_h * ctx

Sharding: each of the 8 cores owns a 1152-row slice of the concatenated
KV axis.  Every core computes (for all 24 (b, head) pairs) the partial
unnormalized numerator  sum_kv exp(s/8)*em_kv*vc  and the partial
denominator sum_kv exp(s/8)*em_kv where em = exp(mask).  The host sums
the partials over cores and divides (flash-attention style combine, no
max subtraction -- scores are O(1) here so exp cannot overflow).

Layouts on device (scores kept transposed [kv, s] throughout):
  qT[b]  [64, 12*768]   head-major, d on partitions
  kv3[b] [128, 6*384]   s-tile major, [k third0..2 | v third0..2] cols
  kcT[b] [64, 1152]     d on partitions, local kv on free
  vca[b] [128, 9*65]    per kv-tile: 64 cols of em-scaled vc + 1 col em
  scoresT psum [128(kv), 768(s)], exp on ACT -> probsT sbuf, ctx via
  lhsT=vca (stationary) x probsT (moving) accumulated over kv tiles.
"""

import numpy as np

B, S, H, NH, D = 2, 768, 768, 12, 64
P = 128
NCORES = 8
KVG = NH * S            # 9216 global kv
KVL = KVG // NCORES     # 1152 local kv per core
NKT = KVL // P          # 9 kv tiles
NT = S // P             # 6 s/H tiles
DP1 = D + 1

_PROGRAM = None
TRACE = False
LAST_RESULTS = None


def _bank_pieces(lo, hi):
    """Split [lo,hi) free-dim range at 512-fp32 PSUM bank boundaries."""
    out = []
    while lo < hi:
        nxt = min(hi, (lo // 512 + 1) * 512)
        out.append((lo, nxt))
        lo = nxt
    return out


def _build_program():
    from contextlib import ExitStack

    import concourse.bacc as bacc
    import concourse.mybir as mybir
    import concourse.tile as tile

    F32 = mybir.dt.float32
    F32R = mybir.dt.float32r
    F16 = mybir.dt.float16
    EXP = mybir.ActivationFunctionType.Exp

    nc = bacc.Bacc("TRN2", target_bir_lowering=False, debug=False,
                   num_devices=NCORES)

    hT = nc.declare_dram_parameter("hT", [B, H, S], F16, isOutput=False)
    wq = nc.declare_dram_parameter("wq", [H, H], F16, isOutput=False)
    bq_d = nc.declare_dram_parameter("bq_d", [P, NH // 2], F32, isOutput=False)
    wkv = nc.declare_dram_parameter("wkv", [H, 6 * D], F16, isOutput=False)
    bkv_d = nc.declare_dram_parameter("bkv_d", [1, 6 * D], F16, isOutput=False)
    mkt = nc.declare_dram_parameter("mkt", [S, KVL], F16, isOutput=False)
    mvt = nc.declare_dram_parameter("mvt", [S, KVL], F16, isOutput=False)
    em_d = nc.declare_dram_parameter("em_d", [B, P, NKT], F32, isOutput=False)
    ones_d = nc.declare_dram_parameter("ones_d", [1, S], F16, isOutput=False)
    out_d = nc.declare_dram_parameter("out_d", [B * NH, DP1, S], F32, isOutput=True)

    with tile.TileContext(nc) as tc, ExitStack() as ctx:
        const = ctx.enter_context(tc.tile_pool(name="const", bufs=1))

        qTa = [const.tile([D, NH * S // 2], F16, name=f"qTa{b}") for b in range(B)]
        qTb = [const.tile([P, NH * S // 2], F16, name=f"qTb{b}") for b in range(B)]
        kv3 = [const.tile([P, NT * 6 * D], F16, name=f"kv3{b}") for b in range(B)]
        kcT = [const.tile([P, KVL], F16, name=f"kcT{b}") for b in range(B)]
        vca = [const.tile([P, NKT * DP1], F16, name=f"vca{b}") for b in range(B)]
        em_s = const.tile([P, B * NKT], F32, name="em_s")
        ones = const.tile([1, S], F16, name="ones")
        bq_s = const.tile([P, NH // 2], F32, name="bq_s")
        bkv_s = const.tile([1, 6 * D], F16, name="bkv_s")

        mkt_s = const.tile([P, NT * KVL], F16, name="mkt_s")
        mvt_s = const.tile([P, NT * KVL], F16, name="mvt_s")
        nc.gpsimd.dma_start(ones[:], ones_d[:])
        for b in range(B):
            nc.gpsimd.dma_start(em_s[:, b * NKT:(b + 1) * NKT], em_d[b])
        nc.gpsimd.dma_start(bq_s[:], bq_d[:])
        nc.gpsimd.dma_start(bkv_s[:], bkv_d[:])

        # ---- Phase A: projections ----
        with tc.tile_pool(name="pa", bufs=1) as pa, \
             tc.tile_pool(name="paps", bufs=2, space="PSUM") as paps:
            wq_s = pa.tile([P, NT * H], F16, name="wq_s")
            wkv_s = pa.tile([P, NT * 6 * D], F16, name="wkv_s")
            hT_tiles = [pa.tile([P, NT * S], F16, name=f"hT_s{b}") for b in range(B)]
            for kt in range(0, NT, 2):
                nc.sync.dma_start(
                    wq_s[:, kt * H:(kt + 2) * H].rearrange("p (kt c) -> p kt c", c=H),
                    wq[kt * P:(kt + 2) * P, :].rearrange("(kt p) c -> p kt c", p=P))
                nc.sync.dma_start(
                    hT_tiles[0][:, kt * S:(kt + 2) * S].rearrange("p (kt c) -> p kt c", c=S),
                    hT[0, kt * P:(kt + 2) * P, :].rearrange("(kt p) c -> p kt c", p=P))
            nc.sync.dma_start(
                wkv_s[:].rearrange("p (kt c) -> p kt c", c=6 * D),
                wkv[:].rearrange("(kt p) c -> p kt c", p=P))
            nc.sync.dma_start(
                hT_tiles[1][:].rearrange("p (kt c) -> p kt c", c=S),
                hT[1].rearrange("(kt p) c -> p kt c", p=P))
            for st in range(NT):
                nc.sync.dma_start(mkt_s[:, st * KVL:(st + 1) * KVL],
                                  mkt[st * P:(st + 1) * P, :])
                nc.sync.dma_start(mvt_s[:, st * KVL:(st + 1) * KVL],
                                  mvt[st * P:(st + 1) * P, :])
            for b in range(B):
                hT_s = hT_tiles[b]
                # qT packed: out [128, 768] = two heads (2t, 2t+1) stacked
                for t in range(NH // 2):
                    q_ps = paps.tile([P, S], F32, name="q_ps", tag="q_ps")
                    for lo, hi in _bank_pieces(0, S):
                        for kt in range(NT):
                            nc.tensor.matmul(
                                q_ps[:, lo:hi],
                                wq_s[:, kt * H + 2 * t * D: kt * H + (2 * t + 2) * D],
                                hT_s[:, kt * S + lo: kt * S + hi],
                                start=(kt == 0), stop=(kt == NT - 1))
                    nc.vector.tensor_scalar_add(
                        qTa[b][:, t * S:(t + 1) * S], q_ps[0:D, :],
                        bq_s[0:D, t:t + 1])
                    nc.vector.tensor_scalar_add(
                        qTb[b][D:P, t * S:(t + 1) * S], q_ps[D:P, :],
                        bq_s[D:P, t:t + 1])
                # k/v thirds: out [128(s), 384] = hT_tile.T @ wkv + ones x bkv
                for st in range(NT):
                    kv_ps = paps.tile([P, 6 * D], F32, name="kv_ps", tag="kv_ps")
                    for kt in range(NT):
                        nc.tensor.matmul(
                            kv_ps[:],
                            hT_s[:, kt * S + st * P: kt * S + (st + 1) * P],
                            wkv_s[:, kt * 6 * D:(kt + 1) * 6 * D],
                            start=(kt == 0), stop=False)
                    nc.tensor.matmul(kv_ps[:], ones[:, 0:P], bkv_s[:],
                                     start=False, stop=True)
                    nc.vector.tensor_copy(kv3[b][:, st * 6 * D:(st + 1) * 6 * D], kv_ps[:])

        # ---- Phase B: memory matmuls ----
        with tc.tile_pool(name="pb", bufs=1) as pb, \
             tc.tile_pool(name="pbps", bufs=2, space="PSUM") as pbps:
            for b in range(B):
                # kcT [128, 1152]: same values in both partition halves so
                # scores lhsT/rhs base partitions can match for odd heads
                kc_ps = pbps.tile([P, KVL], F32, name="kc_ps", tag="kc_ps")
                for j in range(3):
                    for lo, hi in _bank_pieces(384 * j, 384 * (j + 1)):
                        for half, tp in ((0, None), (D, (0, D))):
                            for st in range(NT):
                                nc.tensor.matmul(
                                    kc_ps[half:half + D, lo:hi],
                                    kv3[b][:, st * 6 * D + j * D: st * 6 * D + (j + 1) * D],
                                    mkt_s[:, st * KVL + lo: st * KVL + hi],
                                    start=(st == 0), stop=(st == NT - 1),
                                    tile_position=tp)
                nc.vector.tensor_copy(kcT[b][:], kc_ps[:])
                # vc per kv tile [128, 64], em-scaled into vca
                for t in range(NKT):
                    j = t // 3
                    vc_ps = pbps.tile([P, D], F32, name="vc_ps", tag="vc_ps")
                    for st in range(NT):
                        nc.tensor.matmul(
                            vc_ps[:],
                            mvt_s[:, st * KVL + t * P: st * KVL + (t + 1) * P],
                            kv3[b][:, st * 6 * D + (3 + j) * D: st * 6 * D + (4 + j) * D],
                            start=(st == 0), stop=(st == NT - 1))
                    emc = em_s[:, b * NKT + t: b * NKT + t + 1]
                    nc.vector.tensor_scalar_mul(
                        vca[b][:, t * DP1: t * DP1 + D], vc_ps[:], emc)
                    nc.vector.tensor_copy(
                        vca[b][:, t * DP1 + D: (t + 1) * DP1], emc)

        # ---- Phase C: attention ----
        groups = [(0, 1), (1, 3), (3, 5), (5, 7), (7, 9)]
        with tc.tile_pool(name="pcp", bufs=12) as pcp, \
             tc.tile_pool(name="scps", bufs=2, space="PSUM") as scps, \
             tc.tile_pool(name="ctxps", bufs=1, space="PSUM") as ctxps, \
             tc.tile_pool(name="stg", bufs=2) as stg:
            def emit_scores(b, qh):
                half = D * (qh % 2)
                qsrc = qTa[b] if qh % 2 == 0 else qTb[b]
                qcol = (qh // 2) * S
                prs = []
                for t0, t1 in groups:
                    w = (t1 - t0) * S
                    sc = scps.tile([P, 2 * S], F32, name="sc", tag="sc")
                    pr = pcp.tile([P, 2 * S], F16, name="pr", tag="pr")
                    for t in range(t0, t1):
                        base = (t - t0) * S
                        for lo, hi in _bank_pieces(base, base + S):
                            nc.tensor.matmul(
                                sc[:, lo:hi],
                                kcT[b][half:half + D, t * P:(t + 1) * P],
                                qsrc[half:half + D, qcol + lo - base: qcol + hi - base],
                                start=True, stop=True)
                    nc.scalar.activation(pr[:, 0:w], sc[:, 0:w], EXP, scale=0.125)
                    prs.append((t0, t1, pr))
                return prs

            def emit_ctx(b, pair, prs):
                ctx_ps = ctxps.tile([DP1, S], F32, name="ctx_ps", tag="ctx")
                for t0, t1, pr in prs:
                    for t in range(t0, t1):
                        base = (t - t0) * S
                        for lo, hi in _bank_pieces(0, S):
                            nc.tensor.matmul(
                                ctx_ps[:, lo:hi],
                                vca[b][:, t * DP1:(t + 1) * DP1],
                                pr[:, base + lo: base + hi],
                                start=(t == 0), stop=(t == NKT - 1))
                st_t = stg.tile([DP1, S], F32, name="st_t", tag="st")
                nc.vector.tensor_copy(st_t[:], ctx_ps[:])
                nc.sync.dma_start(out_d[pair], st_t[:])

            # pipeline: scores of pair p+1 are emitted before ctx of pair p,
            # so the ACT engine never starves between pairs
            pending = None
            for b in range(B):
                for qh in range(NH):
                    prs = emit_scores(b, qh)
                    if pending is not None:
                        emit_ctx(*pending)
                    pending = (b, b * NH + qh, prs)
            emit_ctx(*pending)

    nc.compile()
    return nc


def _get_program():
    global _PROGRAM
    if _PROGRAM is None:
        _PROGRAM = _build_program()
    return _PROGRAM


def kernel(hidden_states, attention_mask, Wq, bq, Wk, bk, Wv, bv, gate,
           mem_keys, mem_values):
    from concourse.bass_utils import run_bass_kernel_spmd

    global LAST_RESULTS

    f32 = np.float32
    hidden_states = np.asarray(hidden_states, f32)
    attention_mask = np.asarray(attention_mask, f32)
    Wq = np.asarray(Wq, f32)
    bq = np.asarray(bq, f32)
    Wk = np.asarray(Wk, f32)
    bk = np.asarray(bk, f32)
    Wv = np.asarray(Wv, f32)
    bv = np.asarray(bv, f32)
    gate = np.asarray(gate, f32)
    mem_keys = np.asarray(mem_keys, f32)
    mem_values = np.asarray(mem_values, f32)

    hT = np.ascontiguousarray(hidden_states.transpose(0, 2, 1))
    # MKT[s, h*768+kv] = mem_keys[h, kv, s]
    MKT = np.ascontiguousarray(mem_keys.transpose(2, 0, 1).reshape(S, KVG))
    MVT = np.ascontiguousarray(mem_values.transpose(2, 0, 1).reshape(S, KVG))
    mask = attention_mask.reshape(B, KVG)
    em_full = np.exp(mask).astype(f32)
    hT16 = hT.astype(np.float16)
    Wq16 = Wq.astype(np.float16)

    in_maps = []
    for c in range(NCORES):
        kvb = c * KVL
        heads = [(kvb + 384 * j) // S for j in range(3)]
        wkv_c = np.concatenate(
            [Wk[:, h * D:(h + 1) * D] for h in heads]
            + [Wv[:, h * D:(h + 1) * D] for h in heads], axis=1)
        bkv_c = np.concatenate(
            [bk[h * D:(h + 1) * D] for h in heads]
            + [bv[h * D:(h + 1) * D] for h in heads])[None, :]
        em_c = em_full[:, kvb:kvb + KVL].reshape(B, NKT, P).transpose(0, 2, 1)
        f16 = np.float16
        in_maps.append({
            "hT": hT16,
            "wq": Wq16,
            "bq_d": np.ascontiguousarray(
                bq.reshape(NH // 2, 2, D).transpose(1, 2, 0).reshape(P, NH // 2)),
            "wkv": np.ascontiguousarray(wkv_c).astype(f16),
            "bkv_d": np.ascontiguousarray(bkv_c).astype(f16),
            "mkt": np.ascontiguousarray(MKT[:, kvb:kvb + KVL]).astype(f16),
            "mvt": np.ascontiguousarray(MVT[:, kvb:kvb + KVL]).astype(f16),
            "em_d": np.ascontiguousarray(em_c),
            "ones_d": np.ones((1, S), f16),
        })

    nc = _get_program()
    res = run_bass_kernel_spmd(nc, in_maps, core_ids=list(range(NCORES)),
                               trace=TRACE)
    LAST_RESULTS = res

    parts = res.results[0]["out_d"].astype(f32).copy()
    for rr in res.results[1:]:
        parts += rr["out_d"]
    num = parts[:, :D, :]                      # [24, 64, 768]
    den = parts[:, D, :]                       # [24, 768]
    ctxT = num / den[:, None, :]
    ctx = ctxT.reshape(B, NH, D, S).transpose(0, 3, 1, 2)   # [B, S, NH, D]
    g = (1.0 / (1.0 + np.exp(-gate))).reshape(1, 1, NH, 1)
    return (g * ctx).astype(f32)
